# revision 1
# baseline (speedup 1.0000x reference)
"""Trainium2 kernel for nn_G_MLC_43714177138705 (gnn_message_passing).

Strategy (per sharding hint): data-parallel over the batch dim B across
the 8 NeuronCores — vis_emb is split into 8 shards of 32 batch items;
all parameters, adj, and mask are replicated. Each core runs the full
fused pipeline (rule embedding -> multi-head cross attention -> 10x
two-layer GAT stacks -> class logits -> log_softmax) on its batch
shard, compiled to a single NEFF per core through the Neuron PJRT
backend. The [C, B_shard, K] partial outputs are concatenated on the
batch axis to form the full [C, B, K] output.

Hardcoded problem shapes: B=256, S=64, R=256, V=2000, C=10, K=6, H=4,
D=256 (8 cores -> 32 batch items per core).
"""

import numpy as np
import jax
import jax.numpy as jnp
from functools import partial

B, S, R, V, C, K, H = 256, 64, 256, 2000, 10, 6, 4
D = 256
DH = D // H
NCORES = 8
BL = B // NCORES  # 32 batch items per core
NEG = -1e9


def _gat(h, W, a_s, a_d, b, adj_bias):
    # h: [b,R,Fin] -> [b,R,Fout]; single-head dense GATConv
    hW = h @ W
    e_dst = jnp.einsum('brf,f->br', hW, a_d)
    e_src = jnp.einsum('brf,f->br', hW, a_s)
    e = jax.nn.leaky_relu(e_dst[:, :, None] + e_src[:, None, :], 0.2)
    alpha = jax.nn.softmax(e + adj_bias[None], axis=-1)
    return jnp.einsum('bij,bjf->bif', alpha, hW) + b


def _core_fn(vis_emb, basic, crucial, Wtb, btb, Wtk, btk, Wq, bq, Wk, bk,
             Wv, bv, Wo, bo, W1, a1s, a1d, b1, W2, a2s, a2d, b2, Wl, bl,
             adj, mask):
    # vis_emb: [BL*S, D] shard for this core
    rule = basic @ Wtb + btb + crucial @ Wtk + btk          # [R,D]
    kv = vis_emb.reshape(BL, S, D)
    # Q is batch-independent: rule broadcast across batch
    Q = (rule @ Wq + bq).reshape(R, H, DH)                  # [R,H,DH]
    Kx = (kv @ Wk + bk).reshape(BL, S, H, DH)
    Vx = (kv @ Wv + bv).reshape(BL, S, H, DH)
    att = jnp.einsum('rhd,bshd->bhrs', Q, Kx) / jnp.sqrt(jnp.float32(DH))
    att = jax.nn.softmax(att, axis=-1)
    emb = jnp.einsum('bhrs,bshd->brhd', att, Vx).reshape(BL, R, D) @ Wo + bo

    adj_bias = jnp.where(adj, 0.0, NEG).astype(emb.dtype)   # [R,R]
    outs = []
    for c in range(C):
        h = emb * mask[c].astype(emb.dtype)[None, :, None]
        h = jax.nn.relu(_gat(h, W1[c], a1s[c], a1d[c], b1[c], adj_bias))
        h = _gat(h, W2[c], a2s[c], a2d[c], b2[c], adj_bias)
        h = h @ Wl[c] + bl[c]                               # [BL,R,K]
        outs.append(jax.nn.log_softmax(h.sum(axis=1), axis=1))
    return jnp.stack(outs)                                  # [C,BL,K]


_PARAM_NAMES = ('basic', 'crucial', 'Wtb', 'btb', 'Wtk', 'btk', 'Wq', 'bq',
                'Wk', 'bk', 'Wv', 'bv', 'Wo', 'bo', 'W1', 'a1s', 'a1d', 'b1',
                'W2', 'a2s', 'a2d', 'b2', 'Wl', 'bl', 'adj', 'mask')

_pmapped = jax.pmap(_core_fn, in_axes=(0,) + (None,) * len(_PARAM_NAMES),
                    devices=jax.devices()[:NCORES])


def kernel(**inputs) -> np.ndarray:
    vis = np.ascontiguousarray(inputs['vis_emb']).reshape(NCORES, BL * S, D)
    params = [np.asarray(inputs[n]) for n in _PARAM_NAMES]
    out = _pmapped(vis, *params)                            # [8,C,BL,K]
    out = np.asarray(out)
    # [8,C,BL,K] -> [C, 8*BL, K]
    return np.ascontiguousarray(out.transpose(1, 0, 2, 3).reshape(C, B, K))


if __name__ == '__main__':
    rng = np.random.default_rng(0)
    demo = {
        'vis_emb': rng.standard_normal((B * S, D), dtype=np.float32),
        'basic': (rng.random((R, V)) < 0.01).astype(np.float32),
        'crucial': (rng.random((R, V)) < 0.01).astype(np.float32),
        'adj': rng.random((R, R)) < 0.05,
        'mask': rng.integers(0, 2, (C, R)).astype(np.int32),
    }
    for name, shape in [('Wtb', (V, D)), ('btb', (D,)), ('Wtk', (V, D)),
                        ('btk', (D,)), ('Wq', (D, D)), ('bq', (D,)),
                        ('Wk', (D, D)), ('bk', (D,)), ('Wv', (D, D)),
                        ('bv', (D,)), ('Wo', (D, D)), ('bo', (D,)),
                        ('W1', (C, D, 128)), ('a1s', (C, 128)),
                        ('a1d', (C, 128)), ('b1', (C, 128)),
                        ('W2', (C, 128, 64)), ('a2s', (C, 64)),
                        ('a2d', (C, 64)), ('b2', (C, 64)),
                        ('Wl', (C, 64, K)), ('bl', (C, K))]:
        demo[name] = (rng.standard_normal(shape) * 0.05).astype(np.float32)
    print(kernel(**demo).shape)


# revision 4
# speedup vs baseline: 1.8403x; 1.8403x over previous
"""Trainium2 kernel for nn_G_MLC_43714177138705 (gnn_message_passing).

Strategy (per sharding hint): data-parallel over the batch dim B across
the 8 NeuronCores — vis_emb is split into 8 shards of 32 batch items;
all parameters, adj, and mask are replicated. Each core runs the full
fused pipeline (rule embedding -> multi-head cross attention -> 10x
two-layer GAT stacks -> class logits -> log_softmax) on its batch
shard, compiled to a single NEFF per core through the Neuron PJRT
backend. The [C, B_shard, K] partial outputs are concatenated on the
batch axis to form the full [C, B, K] output.

Hardcoded problem shapes: B=256, S=64, R=256, V=2000, C=10, K=6, H=4,
D=256 (8 cores -> 32 batch items per core).
"""

import numpy as np
import jax
import jax.numpy as jnp
from functools import partial

B, S, R, V, C, K, H = 256, 64, 256, 2000, 10, 6, 4
D = 256
DH = D // H
NCORES = 8
BL = B // NCORES  # 32 batch items per core
NEG = -1e9


def _gat(h, W, a_s, a_d, b, adj_bias):
    # h: [b,R,Fin] -> [b,R,Fout]; single-head dense GATConv
    hW = h @ W
    e_dst = jnp.einsum('brf,f->br', hW, a_d)
    e_src = jnp.einsum('brf,f->br', hW, a_s)
    e = jax.nn.leaky_relu(e_dst[:, :, None] + e_src[:, None, :], 0.2)
    alpha = jax.nn.softmax(e + adj_bias[None], axis=-1)
    return jnp.einsum('bij,bjf->bif', alpha, hW) + b


def _core_fn(vis_emb, rule, Wq, bq, Wk, bk,
             Wv, bv, Wo, bo, W1, a1s, a1d, b1, W2, a2s, a2d, b2, Wl, bl,
             adj, mask):
    # vis_emb: [BL*S, D] shard for this core; rule: [R,D] precomputed
    kv = vis_emb.reshape(BL, S, D)
    # Q is batch-independent: rule broadcast across batch
    Q = (rule @ Wq + bq).reshape(R, H, DH)                  # [R,H,DH]
    Kx = (kv @ Wk + bk).reshape(BL, S, H, DH)
    Vx = (kv @ Wv + bv).reshape(BL, S, H, DH)
    att = jnp.einsum('rhd,bshd->bhrs', Q, Kx) / jnp.sqrt(jnp.float32(DH))
    att = jax.nn.softmax(att, axis=-1)
    emb = jnp.einsum('bhrs,bshd->brhd', att, Vx).reshape(BL, R, D) @ Wo + bo

    adj_bias = jnp.where(adj, 0.0, NEG).astype(emb.dtype)   # [R,R]
    outs = []
    for c in range(C):
        h = emb * mask[c].astype(emb.dtype)[None, :, None]
        h = jax.nn.relu(_gat(h, W1[c], a1s[c], a1d[c], b1[c], adj_bias))
        h = _gat(h, W2[c], a2s[c], a2d[c], b2[c], adj_bias)
        h = h @ Wl[c] + bl[c]                               # [BL,R,K]
        outs.append(jax.nn.log_softmax(h.sum(axis=1), axis=1))
    return jnp.stack(outs)                                  # [C,BL,K]


_PARAM_NAMES = ('Wq', 'bq',
                'Wk', 'bk', 'Wv', 'bv', 'Wo', 'bo', 'W1', 'a1s', 'a1d', 'b1',
                'W2', 'a2s', 'a2d', 'b2', 'Wl', 'bl', 'adj', 'mask')

_pmapped = jax.pmap(_core_fn, in_axes=(0, None) + (None,) * len(_PARAM_NAMES),
                    devices=jax.devices()[:NCORES])


def kernel(**inputs) -> np.ndarray:
    vis = np.ascontiguousarray(inputs['vis_emb']).reshape(NCORES, BL * S, D)
    # rule embedding is batch-independent and tiny [R,D]; computing it on
    # host avoids replicating basic/crucial/Wtb/Wtk (~8MB x 8 cores) to HBM
    rule = (np.asarray(inputs['basic'], np.float32) @ np.asarray(inputs['Wtb'])
            + np.asarray(inputs['btb'])
            + np.asarray(inputs['crucial'], np.float32) @ np.asarray(inputs['Wtk'])
            + np.asarray(inputs['btk'])).astype(np.float32)
    params = [np.asarray(inputs[n]) for n in _PARAM_NAMES]
    out = _pmapped(vis, rule, *params)                      # [8,C,BL,K]
    out = np.asarray(out)
    # [8,C,BL,K] -> [C, 8*BL, K]
    return np.ascontiguousarray(out.transpose(1, 0, 2, 3).reshape(C, B, K))


if __name__ == '__main__':
    rng = np.random.default_rng(0)
    demo = {
        'vis_emb': rng.standard_normal((B * S, D), dtype=np.float32),
        'basic': (rng.random((R, V)) < 0.01).astype(np.float32),
        'crucial': (rng.random((R, V)) < 0.01).astype(np.float32),
        'adj': rng.random((R, R)) < 0.05,
        'mask': rng.integers(0, 2, (C, R)).astype(np.int32),
    }
    for name, shape in [('Wtb', (V, D)), ('btb', (D,)), ('Wtk', (V, D)),
                        ('btk', (D,)), ('Wq', (D, D)), ('bq', (D,)),
                        ('Wk', (D, D)), ('bk', (D,)), ('Wv', (D, D)),
                        ('bv', (D,)), ('Wo', (D, D)), ('bo', (D,)),
                        ('W1', (C, D, 128)), ('a1s', (C, 128)),
                        ('a1d', (C, 128)), ('b1', (C, 128)),
                        ('W2', (C, 128, 64)), ('a2s', (C, 64)),
                        ('a2d', (C, 64)), ('b2', (C, 64)),
                        ('Wl', (C, 64, K)), ('bl', (C, K))]:
        demo[name] = (rng.standard_normal(shape) * 0.05).astype(np.float32)
    print(kernel(**demo).shape)
